# revision 8
# baseline (speedup 1.0000x reference)
"""Trainium2 Bass kernel for nn_BilinearSentenceEncoder.

Computes, for sentence [L=128, B=4096, D=300], size [B], W [D, D]:
  sym-pair scores s_{l+1}^T Wsym s_l (Wsym = (W+W.T)/2), self scores
  s_l^T Wsym s_l, 3-way masked softmax over (prev, self, next) channels,
  and the weighted combination out[l] = w1*s[l] + w0*s[l-1] + w2*s[l+1].

Sharding: data-parallel over B across 8 NeuronCores (512 batch columns
per core); W replicated.  Per-core layout: partition dim = L (=128),
free dims = (b, d).

All-bf16 PE pipeline (v2):
  - S loaded as bf16 via SWDGE cast-DMA (HBM reads stay f32)
  - transposes are REGULAR bf16 matmuls against identity (keeps the PE
    HAM clock-gate warm, pipelines LDWEIGHTS)
  - V/V2 bf16 matmuls; combine matmul bf16 (was fp32 HIGH, ~5x slower)
  - A^T built in bf16 on DVE (2x/4x modes), no f32r re-type DMA
"""

import sys

sys.path.insert(0, "/opt/trn_rl_repo")

import numpy as np
import ml_dtypes

import concourse.bacc as bacc
import concourse.mybir as mybir
from concourse import tile
from concourse.bass_utils import run_bass_kernel_spmd

dt = mybir.dt
AF = mybir.ActivationFunctionType
ALU = mybir.AluOpType

L, B, D = 128, 4096, 300
NCORES = 8
BC = B // NCORES          # 512 batch columns per core
CHUNK = 16                # batch columns per pipeline chunk
NCHUNK = BC // CHUNK      # 32
NEG = np.float32(-1.0e38)
DCH = [(0, 128), (128, 128), (256, 44)]   # d-chunks of 300


def _build_nc(profile=False):
    nc = bacc.Bacc()
    f32, bf16 = dt.float32, dt.bfloat16

    s_in = nc.declare_dram_parameter("s", [L, BC, D], f32, isOutput=False)
    m0_in = nc.declare_dram_parameter("m0", [L, BC], f32, isOutput=False)
    m2_in = nc.declare_dram_parameter("m2", [L, BC], f32, isOutput=False)
    w_in = nc.declare_dram_parameter("wsym", [128, 3 * D], bf16, isOutput=False)
    id_in = nc.declare_dram_parameter("ident", [128, 128], bf16, isOutput=False)
    im_in = nc.declare_dram_parameter("imask", [128, 384], bf16, isOutput=False)
    o_out = nc.declare_dram_parameter("o", [L, BC, D], f32, isOutput=True)

    with tile.TileContext(nc) as tc:
        with (
            tc.tile_pool(name="const", bufs=1) as cpool,
            tc.tile_pool(name="s", bufs=3) as s_pool,
            tc.tile_pool(name="sdn", bufs=3) as sdn_pool,
            tc.tile_pool(name="st", bufs=8) as st_pool,
            tc.tile_pool(name="scr", bufs=1) as scr_pool,
            tc.tile_pool(name="sc", bufs=2) as sc_pool,
            tc.tile_pool(name="atb", bufs=2) as atb_pool,
            tc.tile_pool(name="o", bufs=2) as o_pool,
            tc.tile_pool(name="stp", bufs=2, space="PSUM") as stp_pool,
            tc.tile_pool(name="vp", bufs=2, space="PSUM") as v_pool,
            tc.tile_pool(name="op", bufs=2, space="PSUM") as ops_pool,
        ):
            w_t = cpool.tile([128, 3 * D], bf16)
            id_t = cpool.tile([128, 128], bf16)
            im_t = cpool.tile([128, 384], bf16)
            m0_t = cpool.tile([L, BC], f32)
            m2_t = cpool.tile([L, BC], f32)
            nc.sync.dma_start(out=w_t[:, :], in_=w_in[:, :])
            nc.sync.dma_start(out=id_t[:, :], in_=id_in[:, :])
            nc.sync.dma_start(out=im_t[:, :], in_=im_in[:, :])
            nc.sync.dma_start(out=m0_t[:, :], in_=m0_in[:, :])
            nc.sync.dma_start(out=m2_t[:, :], in_=m2_in[:, :])

            scr = scr_pool.tile([L, D], f32, tag="scr_a")
            scr2 = scr_pool.tile([L, D], f32, tag="scr_b")

            for c in range(NCHUNK):
                b0 = c * CHUNK
                s_t = s_pool.tile([L, CHUNK, D], bf16)
                # SWDGE cast-DMA: HBM f32 -> SBUF bf16
                nc.gpsimd.dma_start(out=s_t[:, :, :], in_=s_in[:, b0 : b0 + CHUNK, :])
                # partition-shifted copy: s_dn[l] = s[l-1]  (row 0 garbage, masked)
                s_dn = sdn_pool.tile([L, CHUNK, D], bf16)
                nc.sync.dma_start(
                    out=s_dn[1:128, :, :], in_=s_t[0:127, :, :]
                )
                # keep row 0 finite (value unused, masked via m0[0]=NEG)
                nc.sync.dma_start(out=s_dn[0:1, :, :], in_=s_t[0:1, :, :])

                a_t = sc_pool.tile([L, CHUNK], f32, tag="a_t")
                symdn = sc_pool.tile([L, CHUNK], f32, tag="symdn")

                for j in range(CHUNK):
                    # transpose via regular bf16 matmuls: ST = s_j.T @ I
                    stp = stp_pool.tile([128, 384], f32)
                    for i, (d0, dn) in enumerate(DCH):
                        nc.tensor.matmul(
                            stp[0:dn, i * 128 : i * 128 + 128],
                            s_t[:, j, d0 : d0 + dn],
                            id_t[:, :],
                            start=True,
                            stop=True,
                        )
                    st = st_pool.tile([128, 384], bf16, tag="st")
                    nc.scalar.activation(st[:, :], stp[:, :], AF.Copy)

                    v = v_pool.tile([128, D], f32)
                    for i, (d0, dn) in enumerate(DCH):
                        nc.tensor.matmul(
                            v[:, :],
                            st[0:dn, i * 128 : i * 128 + 128],
                            w_t[0:dn, i * D : (i + 1) * D],
                            start=(i == 0),
                            stop=(i == 2),
                        )

                    nc.vector.scalar_tensor_tensor(
                        out=scr[:, :],
                        in0=v[:, :],
                        scalar=1.0 / D,
                        in1=s_t[:, j, :],
                        op0=ALU.mult,
                        op1=ALU.mult,
                        accum_out=a_t[:, j : j + 1],
                    )
                    # symdn[l] = sym[l-1] = <V[l], s[l-1]>/D  (row 0 masked)
                    nc.vector.scalar_tensor_tensor(
                        out=scr2[:, :],
                        in0=v[:, :],
                        scalar=1.0 / D,
                        in1=s_dn[:, j, :],
                        op0=ALU.mult,
                        op1=ALU.mult,
                        accum_out=symdn[:, j : j + 1],
                    )

                # ---- chunk softmax (batched over CHUNK columns) ----
                sl = slice(b0, b0 + CHUNK)
                # sym_t[l] = sym[l] = symdn[l+1]; row 127 masked via m2
                sym_t = sc_pool.tile([L, CHUNK], f32, tag="sym_t")
                nc.vector.memset(sym_t[:, :], 0.0)
                nc.sync.dma_start(out=sym_t[0:127, :], in_=symdn[1:128, :])

                l0_t = sc_pool.tile([L, CHUNK], f32, tag="l0")
                l2_t = sc_pool.tile([L, CHUNK], f32, tag="l2")
                nc.vector.tensor_tensor(
                    out=l0_t[:, :], in0=symdn[:, :], in1=m0_t[:, sl], op=ALU.add
                )
                nc.vector.tensor_tensor(
                    out=l2_t[:, :], in0=sym_t[:, :], in1=m2_t[:, sl], op=ALU.add
                )
                e0_t = sc_pool.tile([L, CHUNK], f32, tag="e0")
                e1_t = sc_pool.tile([L, CHUNK], f32, tag="e1")
                e2_t = sc_pool.tile([L, CHUNK], f32, tag="e2")
                nc.scalar.activation(e0_t[:, :], l0_t[:, :], AF.Exp)
                nc.scalar.activation(e1_t[:, :], a_t[:, :], AF.Exp)
                nc.scalar.activation(e2_t[:, :], l2_t[:, :], AF.Exp)
                den_t = sc_pool.tile([L, CHUNK], f32, tag="den")
                nc.vector.tensor_tensor(
                    out=den_t[:, :], in0=e0_t[:, :], in1=e1_t[:, :], op=ALU.add
                )
                nc.vector.tensor_tensor(
                    out=den_t[:, :], in0=den_t[:, :], in1=e2_t[:, :], op=ALU.add
                )
                r_t = sc_pool.tile([L, CHUNK], f32, tag="r")
                nc.vector.reciprocal(r_t[:, :], den_t[:, :])
                w1c = sc_pool.tile([L, CHUNK], f32, tag="w1c")
                w0c = sc_pool.tile([L, CHUNK], f32, tag="w0c")
                w2c = sc_pool.tile([L, CHUNK], f32, tag="w2c")
                nc.vector.tensor_tensor(
                    out=w1c[:, :], in0=e1_t[:, :], in1=r_t[:, :], op=ALU.mult
                )
                nc.vector.tensor_tensor(
                    out=w0c[:, :], in0=e0_t[:, :], in1=r_t[:, :], op=ALU.mult
                )
                nc.vector.tensor_tensor(
                    out=w2c[:, :], in0=e2_t[:, :], in1=r_t[:, :], op=ALU.mult
                )
                w0up = sc_pool.tile([L, CHUNK], f32, tag="w0up")
                w2dn = sc_pool.tile([L, CHUNK], f32, tag="w2dn")
                nc.vector.memset(w0up[:, :], 0.0)
                nc.vector.memset(w2dn[:, :], 0.0)
                nc.sync.dma_start(out=w0up[0:127, :], in_=w0c[1:128, :])
                nc.sync.dma_start(out=w2dn[1:128, :], in_=w2c[0:127, :])

                # ---- build A^T tiles (tridiagonal, bf16) ----
                atb = atb_pool.tile([128, CHUNK * 128], bf16)
                for j in range(CHUNK):
                    av = atb[:, j * 128 : (j + 1) * 128]
                    nc.vector.tensor_scalar(
                        out=av,
                        in0=im_t[:, 0:128],
                        scalar1=w1c[:, j : j + 1],
                        scalar2=None,
                        op0=ALU.mult,
                    )
                    nc.vector.scalar_tensor_tensor(
                        out=av,
                        in0=im_t[:, 128:256],
                        scalar=w0up[:, j : j + 1],
                        in1=av,
                        op0=ALU.mult,
                        op1=ALU.add,
                    )
                    nc.vector.scalar_tensor_tensor(
                        out=av,
                        in0=im_t[:, 256:384],
                        scalar=w2dn[:, j : j + 1],
                        in1=av,
                        op0=ALU.mult,
                        op1=ALU.add,
                    )

                # ---- combine + store ----
                o_t = o_pool.tile([L, CHUNK, D], f32)
                for j in range(CHUNK):
                    ops = ops_pool.tile([128, D], f32)
                    nc.tensor.matmul(
                        ops[:, :],
                        atb[:, j * 128 : (j + 1) * 128],
                        s_t[:, j, :],
                        start=True,
                        stop=True,
                    )
                    nc.scalar.activation(o_t[:, j, :], ops[:, :], AF.Copy)
                nc.sync.dma_start(out=o_out[:, b0 : b0 + CHUNK, :], in_=o_t[:, :, :])

    nc.compile()
    return nc


_NC_CACHE = {}


def _get_nc():
    if "nc" not in _NC_CACHE:
        _NC_CACHE["nc"] = _build_nc()
    return _NC_CACHE["nc"]


def _host_inputs(sentence, size, W):
    sentence = np.ascontiguousarray(np.asarray(sentence, dtype=np.float32))
    size = np.asarray(size).astype(np.int64)
    W = np.asarray(W, dtype=np.float32)

    wsym = 0.5 * (W + W.T)
    w_pack = np.zeros((128, 3 * D), dtype=ml_dtypes.bfloat16)
    for i, (d0, dn) in enumerate(DCH):
        w_pack[0:dn, i * D : (i + 1) * D] = wsym[d0 : d0 + dn, :].astype(
            ml_dtypes.bfloat16
        )

    ident = np.eye(128, dtype=ml_dtypes.bfloat16)
    I0 = np.eye(128, dtype=np.float32)
    Iup = np.zeros((128, 128), np.float32)
    Iup[np.arange(127), np.arange(1, 128)] = 1.0
    Idn = np.zeros((128, 128), np.float32)
    Idn[np.arange(1, 128), np.arange(127)] = 1.0
    imask = np.ascontiguousarray(
        np.concatenate([I0, Iup, Idn], axis=1).astype(ml_dtypes.bfloat16)
    )

    pos = np.arange(L, dtype=np.int64)[:, None]
    m0 = np.where(pos < size[None, :], 0.0, NEG).astype(np.float32)
    m0[0, :] = NEG
    m2 = np.where(pos < np.clip(size - 1, 0, None)[None, :], 0.0, NEG).astype(
        np.float32
    )
    m2[L - 1, :] = NEG

    in_maps = []
    for c in range(NCORES):
        bsl = slice(c * BC, (c + 1) * BC)
        in_maps.append(
            {
                "s": np.ascontiguousarray(sentence[:, bsl, :]),
                "m0": np.ascontiguousarray(m0[:, bsl]),
                "m2": np.ascontiguousarray(m2[:, bsl]),
                "wsym": w_pack,
                "ident": ident,
                "imask": imask,
            }
        )
    return in_maps


def kernel(sentence, size, W):
    nc = _get_nc()
    in_maps = _host_inputs(sentence, size, W)
    res = run_bass_kernel_spmd(nc, in_maps, core_ids=list(range(NCORES)))
    out = np.concatenate([res.results[c]["o"] for c in range(NCORES)], axis=1)
    return out.astype(np.float32)


def _install_ntff_hook():
    """Register the axon NTFF profiling hook that this container's boot
    skipped (antenv.axon_hooks module absent). Mirrors
    trn_agent_boot/trn_boot.py:_ntff_profile_via_ctypes."""
    try:
        from antenv.axon_hooks import get_axon_ntff_profile_hook  # noqa: F401

        return
    except ImportError:
        pass
    import contextlib
    import ctypes
    import types

    so_path = "/opt/axon/libaxon_pjrt.so"
    lib = ctypes.CDLL(so_path)
    if not hasattr(lib, "axon_start_nrt_profile"):
        return
    lib.axon_start_nrt_profile.argtypes = [
        ctypes.POINTER(ctypes.c_int64),
        ctypes.c_size_t,
    ]
    lib.axon_start_nrt_profile.restype = ctypes.c_int64
    lib.axon_stop_nrt_profile.argtypes = [ctypes.c_char_p]
    lib.axon_stop_nrt_profile.restype = ctypes.c_int64

    @contextlib.contextmanager
    def _hook(output_dir, device_ids):
        import jax

        jax.devices()
        if device_ids:
            ids = (ctypes.c_int64 * len(device_ids))(*device_ids)
            rc = lib.axon_start_nrt_profile(ids, len(device_ids))
        else:
            rc = lib.axon_start_nrt_profile(None, 0)
        if rc != 0:
            raise RuntimeError(f"axon_start_nrt_profile rc={rc}")
        try:
            yield
        finally:
            n = lib.axon_stop_nrt_profile(str(output_dir).encode())
            print(f"ntff capture: {n} file(s) -> {output_dir}")

    mod = types.ModuleType("antenv.axon_hooks")
    mod.get_axon_ntff_profile_hook = lambda: _hook
    mod.set_axon_ntff_profile_hook = lambda h: None
    import antenv

    sys.modules["antenv.axon_hooks"] = mod
    antenv.axon_hooks = mod


def run_traced(sentence, size, W):
    """Like kernel(), but also returns (exec_time_ns, profile_json path)."""
    _install_ntff_hook()
    nc = _get_nc()
    in_maps = _host_inputs(sentence, size, W)
    res = run_bass_kernel_spmd(
        nc, in_maps, core_ids=list(range(NCORES)), trace=True, trace_cores=[0]
    )
    out = np.concatenate([res.results[c]["o"] for c in range(NCORES)], axis=1)
    return out.astype(np.float32), res.exec_time_ns, res.profile_json


if __name__ == "__main__":
    rng = np.random.default_rng(0)
    s = rng.standard_normal((L, B, D)).astype(np.float32)
    sz = rng.integers(0, L, size=(B,)).astype(np.int32)
    W = (rng.standard_normal((D, D)) / np.sqrt(D)).astype(np.float32)
    out = kernel(s, sz, W)
    print("out", out.shape, out.dtype, np.abs(out).max())


# revision 11
# speedup vs baseline: 1.6710x; 1.6710x over previous
"""Trainium2 Bass kernel for nn_BilinearSentenceEncoder.

Computes, for sentence [L=128, B=4096, D=300], size [B], W [D, D]:
  sym-pair scores s_{l+1}^T Wsym s_l (Wsym = (W+W.T)/2), self scores
  s_l^T Wsym s_l, 3-way masked softmax over (prev, self, next) channels,
  and the weighted combination out[l] = w1*s[l] + w0*s[l-1] + w2*s[l+1].

Sharding: data-parallel over B across 8 NeuronCores (512 batch columns
per core); W replicated.  Per-core layout: partition dim = L (=128),
free dims = (b, d).

All-bf16 PE pipeline (v2):
  - S loaded as bf16 via SWDGE cast-DMA (HBM reads stay f32)
  - transposes are REGULAR bf16 matmuls against identity (keeps the PE
    HAM clock-gate warm, pipelines LDWEIGHTS)
  - V/V2 bf16 matmuls; combine matmul bf16 (was fp32 HIGH, ~5x slower)
  - A^T built in bf16 on DVE (2x/4x modes), no f32r re-type DMA
"""

import sys

sys.path.insert(0, "/opt/trn_rl_repo")

import numpy as np
import ml_dtypes

import concourse.bacc as bacc
import concourse.mybir as mybir
from concourse import tile
from concourse.bass_utils import run_bass_kernel_spmd

dt = mybir.dt
AF = mybir.ActivationFunctionType
ALU = mybir.AluOpType

L, B, D = 128, 4096, 300
NCORES = 8
BC = B // NCORES          # 512 batch columns per core
CHUNK = 16                # batch columns per pipeline chunk
NCHUNK = BC // CHUNK      # 32
NEG = np.float32(-1.0e38)
DCH = [(0, 128), (128, 128), (256, 44)]   # d-chunks of 300


def _build_nc(profile=False):
    nc = bacc.Bacc()
    f32, bf16 = dt.float32, dt.bfloat16

    s_in = nc.declare_dram_parameter("s", [L, BC, D], f32, isOutput=False)
    m0_in = nc.declare_dram_parameter("m0", [L, BC], f32, isOutput=False)
    m2_in = nc.declare_dram_parameter("m2", [L, BC], f32, isOutput=False)
    w_in = nc.declare_dram_parameter("wsym", [128, 3 * D], bf16, isOutput=False)
    id_in = nc.declare_dram_parameter("ident", [128, 128], bf16, isOutput=False)
    im_in = nc.declare_dram_parameter("imask", [128, 384], bf16, isOutput=False)
    o_out = nc.declare_dram_parameter("o", [L, BC, D], f32, isOutput=True)

    with tile.TileContext(nc) as tc:
        with (
            tc.tile_pool(name="const", bufs=1) as cpool,
            tc.tile_pool(name="s", bufs=3) as s_pool,
            tc.tile_pool(name="sdn", bufs=3) as sdn_pool,
            tc.tile_pool(name="st", bufs=8) as st_pool,
            tc.tile_pool(name="scr", bufs=1) as scr_pool,
            tc.tile_pool(name="sc", bufs=2) as sc_pool,
            tc.tile_pool(name="atb", bufs=2) as atb_pool,
            tc.tile_pool(name="o", bufs=2) as o_pool,
            tc.tile_pool(name="stp", bufs=2, space="PSUM") as stp_pool,
            tc.tile_pool(name="vp", bufs=2, space="PSUM") as v_pool,
            tc.tile_pool(name="op", bufs=2, space="PSUM") as ops_pool,
        ):
            w_t = cpool.tile([128, 3 * D], bf16)
            id_t = cpool.tile([128, 128], bf16)
            im_t = cpool.tile([128, 384], bf16)
            m0_t = cpool.tile([L, BC], f32)
            m2_t = cpool.tile([L, BC], f32)
            nc.sync.dma_start(out=w_t[:, :], in_=w_in[:, :])
            nc.sync.dma_start(out=id_t[:, :], in_=id_in[:, :])
            nc.sync.dma_start(out=im_t[:, :], in_=im_in[:, :])
            nc.sync.dma_start(out=m0_t[:, :], in_=m0_in[:, :])
            nc.sync.dma_start(out=m2_t[:, :], in_=m2_in[:, :])

            scr = scr_pool.tile([L, D], f32, tag="scr_a")
            scr2 = scr_pool.tile([L, D], f32, tag="scr_b")

            for c in range(NCHUNK):
                b0 = c * CHUNK
                s_t = s_pool.tile([L, CHUNK, D], bf16)
                # SWDGE cast-DMA: HBM f32 -> SBUF bf16
                nc.gpsimd.dma_start(out=s_t[:, :, :], in_=s_in[:, b0 : b0 + CHUNK, :])
                # partition-shifted copy: s_dn[l] = s[l-1]  (row 0 garbage, masked).
                # Issued as 8 partition-range DMAs: a single shifted SBUF->SBUF
                # DMA can't fan out across SDMA engines (lands on one engine
                # at ~27 GiB/s); 8 separate calls parallelize.
                s_dn = sdn_pool.tile([L, CHUNK, D], bf16)
                nc.sync.dma_start(out=s_dn[0:1, :, :], in_=s_t[0:1, :, :])
                for p0 in range(0, 128, 16):
                    pd0, pd1 = max(p0, 1), p0 + 16
                    nc.sync.dma_start(
                        out=s_dn[pd0:pd1, :, :], in_=s_t[pd0 - 1 : pd1 - 1, :, :]
                    )

                a_t = sc_pool.tile([L, CHUNK], f32, tag="a_t")
                symdn = sc_pool.tile([L, CHUNK], f32, tag="symdn")

                for j in range(CHUNK):
                    # transpose via regular bf16 matmuls: ST = s_j.T @ I
                    stp = stp_pool.tile([128, 384], f32)
                    for i, (d0, dn) in enumerate(DCH):
                        nc.tensor.matmul(
                            stp[0:dn, i * 128 : i * 128 + 128],
                            s_t[:, j, d0 : d0 + dn],
                            id_t[:, :],
                            start=True,
                            stop=True,
                        )
                    st = st_pool.tile([128, 384], bf16, tag="st")
                    nc.scalar.activation(st[:, :], stp[:, :], AF.Copy)

                    v = v_pool.tile([128, D], f32)
                    for i, (d0, dn) in enumerate(DCH):
                        nc.tensor.matmul(
                            v[:, :],
                            st[0:dn, i * 128 : i * 128 + 128],
                            w_t[0:dn, i * D : (i + 1) * D],
                            start=(i == 0),
                            stop=(i == 2),
                        )

                    nc.vector.scalar_tensor_tensor(
                        out=scr[:, :],
                        in0=v[:, :],
                        scalar=1.0 / D,
                        in1=s_t[:, j, :],
                        op0=ALU.mult,
                        op1=ALU.mult,
                        accum_out=a_t[:, j : j + 1],
                    )
                    # symdn[l] = sym[l-1] = <V[l], s[l-1]>/D  (row 0 masked)
                    nc.vector.scalar_tensor_tensor(
                        out=scr2[:, :],
                        in0=v[:, :],
                        scalar=1.0 / D,
                        in1=s_dn[:, j, :],
                        op0=ALU.mult,
                        op1=ALU.mult,
                        accum_out=symdn[:, j : j + 1],
                    )

                # ---- chunk softmax (batched over CHUNK columns) ----
                sl = slice(b0, b0 + CHUNK)
                # sym_t[l] = sym[l] = symdn[l+1]; row 127 masked via m2
                sym_t = sc_pool.tile([L, CHUNK], f32, tag="sym_t")
                nc.vector.memset(sym_t[:, :], 0.0)
                nc.sync.dma_start(out=sym_t[0:127, :], in_=symdn[1:128, :])

                l0_t = sc_pool.tile([L, CHUNK], f32, tag="l0")
                l2_t = sc_pool.tile([L, CHUNK], f32, tag="l2")
                nc.vector.tensor_tensor(
                    out=l0_t[:, :], in0=symdn[:, :], in1=m0_t[:, sl], op=ALU.add
                )
                nc.vector.tensor_tensor(
                    out=l2_t[:, :], in0=sym_t[:, :], in1=m2_t[:, sl], op=ALU.add
                )
                e0_t = sc_pool.tile([L, CHUNK], f32, tag="e0")
                e1_t = sc_pool.tile([L, CHUNK], f32, tag="e1")
                e2_t = sc_pool.tile([L, CHUNK], f32, tag="e2")
                nc.scalar.activation(e0_t[:, :], l0_t[:, :], AF.Exp)
                nc.scalar.activation(e1_t[:, :], a_t[:, :], AF.Exp)
                nc.scalar.activation(e2_t[:, :], l2_t[:, :], AF.Exp)
                den_t = sc_pool.tile([L, CHUNK], f32, tag="den")
                nc.vector.tensor_tensor(
                    out=den_t[:, :], in0=e0_t[:, :], in1=e1_t[:, :], op=ALU.add
                )
                nc.vector.tensor_tensor(
                    out=den_t[:, :], in0=den_t[:, :], in1=e2_t[:, :], op=ALU.add
                )
                r_t = sc_pool.tile([L, CHUNK], f32, tag="r")
                nc.vector.reciprocal(r_t[:, :], den_t[:, :])
                w1c = sc_pool.tile([L, CHUNK], f32, tag="w1c")
                w0c = sc_pool.tile([L, CHUNK], f32, tag="w0c")
                w2c = sc_pool.tile([L, CHUNK], f32, tag="w2c")
                nc.vector.tensor_tensor(
                    out=w1c[:, :], in0=e1_t[:, :], in1=r_t[:, :], op=ALU.mult
                )
                nc.vector.tensor_tensor(
                    out=w0c[:, :], in0=e0_t[:, :], in1=r_t[:, :], op=ALU.mult
                )
                nc.vector.tensor_tensor(
                    out=w2c[:, :], in0=e2_t[:, :], in1=r_t[:, :], op=ALU.mult
                )
                w0up = sc_pool.tile([L, CHUNK], f32, tag="w0up")
                w2dn = sc_pool.tile([L, CHUNK], f32, tag="w2dn")
                nc.vector.memset(w0up[:, :], 0.0)
                nc.vector.memset(w2dn[:, :], 0.0)
                nc.sync.dma_start(out=w0up[0:127, :], in_=w0c[1:128, :])
                nc.sync.dma_start(out=w2dn[1:128, :], in_=w2c[0:127, :])

                # ---- build A^T tiles (tridiagonal, bf16) ----
                # Whole chunk in 5 broadcast-AP TT ops: mask [128,128]
                # broadcast over j (stride 0), weights [128,CHUNK] broadcast
                # over l (stride 0).
                atb = atb_pool.tile([128, CHUNK, 128], bf16)
                atmp = atb_pool.tile([128, CHUNK, 128], bf16, tag="atmp")
                im0b = im_t[:, 0:128].unsqueeze(1).broadcast_to([128, CHUNK, 128])
                imub = im_t[:, 128:256].unsqueeze(1).broadcast_to([128, CHUNK, 128])
                imdb = im_t[:, 256:384].unsqueeze(1).broadcast_to([128, CHUNK, 128])
                w1b = w1c[:, :].unsqueeze(2).broadcast_to([128, CHUNK, 128])
                w0b = w0up[:, :].unsqueeze(2).broadcast_to([128, CHUNK, 128])
                w2b = w2dn[:, :].unsqueeze(2).broadcast_to([128, CHUNK, 128])
                nc.vector.tensor_tensor(
                    out=atb[:, :, :], in0=im0b, in1=w1b, op=ALU.mult
                )
                nc.vector.tensor_tensor(
                    out=atmp[:, :, :], in0=imub, in1=w0b, op=ALU.mult
                )
                nc.vector.tensor_tensor(
                    out=atb[:, :, :], in0=atb[:, :, :], in1=atmp[:, :, :], op=ALU.add
                )
                nc.vector.tensor_tensor(
                    out=atmp[:, :, :], in0=imdb, in1=w2b, op=ALU.mult
                )
                nc.vector.tensor_tensor(
                    out=atb[:, :, :], in0=atb[:, :, :], in1=atmp[:, :, :], op=ALU.add
                )

                # ---- combine + store ----
                o_t = o_pool.tile([L, CHUNK, D], f32)
                for j in range(CHUNK):
                    ops = ops_pool.tile([128, D], f32)
                    nc.tensor.matmul(
                        ops[:, :],
                        atb[:, j, :],
                        s_t[:, j, :],
                        start=True,
                        stop=True,
                    )
                    nc.scalar.activation(o_t[:, j, :], ops[:, :], AF.Copy)
                nc.sync.dma_start(out=o_out[:, b0 : b0 + CHUNK, :], in_=o_t[:, :, :])

    nc.compile()
    return nc


_NC_CACHE = {}


def _get_nc():
    if "nc" not in _NC_CACHE:
        _NC_CACHE["nc"] = _build_nc()
    return _NC_CACHE["nc"]


def _host_inputs(sentence, size, W):
    sentence = np.ascontiguousarray(np.asarray(sentence, dtype=np.float32))
    size = np.asarray(size).astype(np.int64)
    W = np.asarray(W, dtype=np.float32)

    wsym = 0.5 * (W + W.T)
    w_pack = np.zeros((128, 3 * D), dtype=ml_dtypes.bfloat16)
    for i, (d0, dn) in enumerate(DCH):
        w_pack[0:dn, i * D : (i + 1) * D] = wsym[d0 : d0 + dn, :].astype(
            ml_dtypes.bfloat16
        )

    ident = np.eye(128, dtype=ml_dtypes.bfloat16)
    I0 = np.eye(128, dtype=np.float32)
    Iup = np.zeros((128, 128), np.float32)
    Iup[np.arange(127), np.arange(1, 128)] = 1.0
    Idn = np.zeros((128, 128), np.float32)
    Idn[np.arange(1, 128), np.arange(127)] = 1.0
    imask = np.ascontiguousarray(
        np.concatenate([I0, Iup, Idn], axis=1).astype(ml_dtypes.bfloat16)
    )

    pos = np.arange(L, dtype=np.int64)[:, None]
    m0 = np.where(pos < size[None, :], 0.0, NEG).astype(np.float32)
    m0[0, :] = NEG
    m2 = np.where(pos < np.clip(size - 1, 0, None)[None, :], 0.0, NEG).astype(
        np.float32
    )
    m2[L - 1, :] = NEG

    in_maps = []
    for c in range(NCORES):
        bsl = slice(c * BC, (c + 1) * BC)
        in_maps.append(
            {
                "s": np.ascontiguousarray(sentence[:, bsl, :]),
                "m0": np.ascontiguousarray(m0[:, bsl]),
                "m2": np.ascontiguousarray(m2[:, bsl]),
                "wsym": w_pack,
                "ident": ident,
                "imask": imask,
            }
        )
    return in_maps


def kernel(sentence, size, W):
    nc = _get_nc()
    in_maps = _host_inputs(sentence, size, W)
    res = run_bass_kernel_spmd(nc, in_maps, core_ids=list(range(NCORES)))
    out = np.concatenate([res.results[c]["o"] for c in range(NCORES)], axis=1)
    return out.astype(np.float32)


def _install_ntff_hook():
    """Register the axon NTFF profiling hook that this container's boot
    skipped (antenv.axon_hooks module absent). Mirrors
    trn_agent_boot/trn_boot.py:_ntff_profile_via_ctypes."""
    try:
        from antenv.axon_hooks import get_axon_ntff_profile_hook  # noqa: F401

        return
    except ImportError:
        pass
    import contextlib
    import ctypes
    import types

    so_path = "/opt/axon/libaxon_pjrt.so"
    lib = ctypes.CDLL(so_path)
    if not hasattr(lib, "axon_start_nrt_profile"):
        return
    lib.axon_start_nrt_profile.argtypes = [
        ctypes.POINTER(ctypes.c_int64),
        ctypes.c_size_t,
    ]
    lib.axon_start_nrt_profile.restype = ctypes.c_int64
    lib.axon_stop_nrt_profile.argtypes = [ctypes.c_char_p]
    lib.axon_stop_nrt_profile.restype = ctypes.c_int64

    @contextlib.contextmanager
    def _hook(output_dir, device_ids):
        import jax

        jax.devices()
        if device_ids:
            ids = (ctypes.c_int64 * len(device_ids))(*device_ids)
            rc = lib.axon_start_nrt_profile(ids, len(device_ids))
        else:
            rc = lib.axon_start_nrt_profile(None, 0)
        if rc != 0:
            raise RuntimeError(f"axon_start_nrt_profile rc={rc}")
        try:
            yield
        finally:
            n = lib.axon_stop_nrt_profile(str(output_dir).encode())
            print(f"ntff capture: {n} file(s) -> {output_dir}")

    mod = types.ModuleType("antenv.axon_hooks")
    mod.get_axon_ntff_profile_hook = lambda: _hook
    mod.set_axon_ntff_profile_hook = lambda h: None
    import antenv

    sys.modules["antenv.axon_hooks"] = mod
    antenv.axon_hooks = mod


def run_traced(sentence, size, W):
    """Like kernel(), but also returns (exec_time_ns, profile_json path)."""
    _install_ntff_hook()
    nc = _get_nc()
    in_maps = _host_inputs(sentence, size, W)
    res = run_bass_kernel_spmd(
        nc, in_maps, core_ids=list(range(NCORES)), trace=True, trace_cores=[0]
    )
    out = np.concatenate([res.results[c]["o"] for c in range(NCORES)], axis=1)
    return out.astype(np.float32), res.exec_time_ns, res.profile_json


if __name__ == "__main__":
    rng = np.random.default_rng(0)
    s = rng.standard_normal((L, B, D)).astype(np.float32)
    sz = rng.integers(0, L, size=(B,)).astype(np.int32)
    W = (rng.standard_normal((D, D)) / np.sqrt(D)).astype(np.float32)
    out = kernel(s, sz, W)
    print("out", out.shape, out.dtype, np.abs(out).max())


# revision 15
# speedup vs baseline: 1.7487x; 1.0464x over previous
"""Trainium2 Bass kernel for nn_BilinearSentenceEncoder.

Computes, for sentence [L=128, B=4096, D=300], size [B], W [D, D]:
  sym-pair scores s_{l+1}^T Wsym s_l (Wsym = (W+W.T)/2), self scores
  s_l^T Wsym s_l, 3-way masked softmax over (prev, self, next) channels,
  and the weighted combination out[l] = w1*s[l] + w0*s[l-1] + w2*s[l+1].

Sharding: data-parallel over B across 8 NeuronCores (512 batch columns
per core); W replicated.  Per-core layout: partition dim = L (=128),
free dims = (b, d).

All-bf16 PE pipeline (v2):
  - S loaded as bf16 via SWDGE cast-DMA (HBM reads stay f32)
  - transposes are REGULAR bf16 matmuls against identity (keeps the PE
    HAM clock-gate warm, pipelines LDWEIGHTS)
  - V/V2 bf16 matmuls; combine matmul bf16 (was fp32 HIGH, ~5x slower)
  - A^T built in bf16 on DVE (2x/4x modes), no f32r re-type DMA
"""

import sys

sys.path.insert(0, "/opt/trn_rl_repo")

import numpy as np
import ml_dtypes

import concourse.bacc as bacc
import concourse.mybir as mybir
from concourse import tile
from concourse.bass_utils import run_bass_kernel_spmd

dt = mybir.dt
AF = mybir.ActivationFunctionType
ALU = mybir.AluOpType

L, B, D = 128, 4096, 300
NCORES = 8
BC = B // NCORES          # 512 batch columns per core
CHUNK = 16                # batch columns per pipeline chunk
NCHUNK = BC // CHUNK      # 32
NEG = np.float32(-1.0e38)
DCH = [(0, 128), (128, 128), (256, 44)]   # d-chunks of 300
DP = 304                  # rhs free padded to %16 for DoubleRow


def _build_nc(profile=False):
    nc = bacc.Bacc()
    f32, bf16, fp8 = dt.float32, dt.bfloat16, dt.float8e4

    s_in = nc.declare_dram_parameter("s", [L, BC, D], f32, isOutput=False)
    m0_in = nc.declare_dram_parameter("m0", [L, BC], f32, isOutput=False)
    m2_in = nc.declare_dram_parameter("m2", [L, BC], f32, isOutput=False)
    wdr_in = nc.declare_dram_parameter("wdr", [128, 2, DP], fp8, isOutput=False)
    wtl_in = nc.declare_dram_parameter("wtl", [44, DP], fp8, isOutput=False)
    id_in = nc.declare_dram_parameter("ident", [128, 128], bf16, isOutput=False)
    im_in = nc.declare_dram_parameter("imask", [128, 384], dt.uint8, isOutput=False)
    o_out = nc.declare_dram_parameter("o", [L, BC, D], f32, isOutput=True)

    with tile.TileContext(nc) as tc:
        with (
            tc.tile_pool(name="const", bufs=1) as cpool,
            tc.tile_pool(name="s", bufs=3) as s_pool,
            tc.tile_pool(name="sdn", bufs=3) as sdn_pool,
            tc.tile_pool(name="st", bufs=8) as st_pool,
            tc.tile_pool(name="scr", bufs=1) as scr_pool,
            tc.tile_pool(name="sc", bufs=2) as sc_pool,
            tc.tile_pool(name="atb", bufs=2) as atb_pool,
            tc.tile_pool(name="o", bufs=2) as o_pool,
            tc.tile_pool(name="stp", bufs=3, space="PSUM") as stp_pool,
            tc.tile_pool(name="vp", bufs=3, space="PSUM") as v_pool,
            tc.tile_pool(name="op", bufs=2, space="PSUM") as ops_pool,
        ):
            wdr_t = cpool.tile([128, 2, DP], fp8)
            wtl_t = cpool.tile([44, DP], fp8)
            id_t = cpool.tile([128, 128], bf16)
            im_t = cpool.tile([128, 384], dt.uint8)
            m0_t = cpool.tile([L, BC], f32)
            m2_t = cpool.tile([L, BC], f32)
            nc.sync.dma_start(out=wdr_t[:, :, :], in_=wdr_in[:, :, :])
            nc.sync.dma_start(out=wtl_t[:, :], in_=wtl_in[:, :])
            nc.sync.dma_start(out=id_t[:, :], in_=id_in[:, :])
            nc.sync.dma_start(out=im_t[:, :], in_=im_in[:, :])
            nc.sync.dma_start(out=m0_t[:, :], in_=m0_in[:, :])
            nc.sync.dma_start(out=m2_t[:, :], in_=m2_in[:, :])

            scr = scr_pool.tile([L, D], f32, tag="scr_a")
            scr2 = scr_pool.tile([L, D], f32, tag="scr_b")

            prev = None  # (b0, s_t, w1c, w0up, w2dn) of previous chunk

            def emit_combine(pstate):
                # A^T build + combine + store for a finished chunk; emitted
                # one chunk late so DVE never head-of-line blocks on the
                # softmax DMA shifts.
                pb0, ps_t, pw1c, pw0up, pw2dn, pc = pstate
                atb = atb_pool.tile([128, CHUNK, 128], bf16)
                if pc < 2:
                    nc.vector.memset(atb[:, :, :], 0.0)
                im0b = im_t[:, 0:128].unsqueeze(1).broadcast_to([128, CHUNK, 128])
                imub = im_t[:, 128:256].unsqueeze(1).broadcast_to([128, CHUNK, 128])
                imdb = im_t[:, 256:384].unsqueeze(1).broadcast_to([128, CHUNK, 128])
                w1b = pw1c[:, :].unsqueeze(2).broadcast_to([128, CHUNK, 128])
                w0b = pw0up[:, :].unsqueeze(2).broadcast_to([128, CHUNK, 128])
                w2b = pw2dn[:, :].unsqueeze(2).broadcast_to([128, CHUNK, 128])
                nc.vector.copy_predicated(out=atb[:, :, :], mask=im0b, data=w1b)
                nc.vector.copy_predicated(out=atb[:, :, :], mask=imub, data=w0b)
                nc.vector.copy_predicated(out=atb[:, :, :], mask=imdb, data=w2b)
                o_t = o_pool.tile([L, CHUNK, D], f32)
                for j in range(CHUNK):
                    ops = ops_pool.tile([128, D], f32)
                    nc.tensor.matmul(
                        ops[:, :],
                        atb[:, j, :],
                        ps_t[:, j, :],
                        start=True,
                        stop=True,
                    )
                    nc.scalar.activation(o_t[:, j, :], ops[:, :], AF.Copy)
                nc.sync.dma_start(out=o_out[:, pb0 : pb0 + CHUNK, :], in_=o_t[:, :, :])

            for c in range(NCHUNK):
                b0 = c * CHUNK
                s_t = s_pool.tile([L, CHUNK, D], bf16)
                # SWDGE cast-DMA: HBM f32 -> SBUF bf16
                nc.gpsimd.dma_start(out=s_t[:, :, :], in_=s_in[:, b0 : b0 + CHUNK, :])
                # partition-shifted copy: s_dn[l] = s[l-1]  (row 0 garbage, masked).
                # Issued as 8 partition-range DMAs: a single shifted SBUF->SBUF
                # DMA can't fan out across SDMA engines (lands on one engine
                # at ~27 GiB/s); 8 separate calls parallelize.
                s_dn = sdn_pool.tile([L, CHUNK, D], bf16)
                nc.sync.dma_start(out=s_dn[0:1, :, :], in_=s_t[0:1, :, :])
                for p0 in range(0, 128, 16):
                    pd0, pd1 = max(p0, 1), p0 + 16
                    nc.sync.dma_start(
                        out=s_dn[pd0:pd1, :, :], in_=s_t[pd0 - 1 : pd1 - 1, :, :]
                    )

                a_t = sc_pool.tile([L, CHUNK], f32, tag="a_t")
                symdn = sc_pool.tile([L, CHUNK], f32, tag="symdn")

                for j in range(CHUNK):
                    # transpose via regular bf16 matmuls: ST = s_j.T @ I
                    stp = stp_pool.tile([128, 384], f32)
                    for i, (d0, dn) in enumerate(DCH):
                        nc.tensor.matmul(
                            stp[0:dn, i * 128 : i * 128 + 128],
                            s_t[:, j, d0 : d0 + dn],
                            id_t[:, :],
                            start=True,
                            stop=True,
                        )
                    st = st_pool.tile([128, 3, 128], fp8, tag="st")
                    nc.scalar.activation(st[:, :, :], stp[:, :], AF.Copy)

                    v = v_pool.tile([128, DP], f32)
                    nc.tensor.matmul(
                        v[:, :],
                        st[:, 0:2, :],
                        wdr_t[:, :, :],
                        start=True,
                        stop=False,
                        perf_mode=mybir.MatmulPerfMode.DoubleRow,
                    )
                    nc.tensor.matmul(
                        v[:, :],
                        st[0:44, 2, :],
                        wtl_t[0:44, :],
                        start=False,
                        stop=True,
                    )

                    nc.vector.scalar_tensor_tensor(
                        out=scr[:, :],
                        in0=v[:, 0:D],
                        scalar=1.0 / (16.0 * D),
                        in1=s_t[:, j, :],
                        op0=ALU.mult,
                        op1=ALU.mult,
                        accum_out=a_t[:, j : j + 1],
                    )
                    # symdn[l] = sym[l-1] = <V[l], s[l-1]>/D  (row 0 masked)
                    nc.vector.scalar_tensor_tensor(
                        out=scr2[:, :],
                        in0=v[:, 0:D],
                        scalar=1.0 / (16.0 * D),
                        in1=s_dn[:, j, :],
                        op0=ALU.mult,
                        op1=ALU.mult,
                        accum_out=symdn[:, j : j + 1],
                    )

                # ---- chunk softmax (batched over CHUNK columns) ----
                sl = slice(b0, b0 + CHUNK)
                # sym_t[l] = sym[l] = symdn[l+1]; row 127 masked via m2
                sym_t = sc_pool.tile([L, CHUNK], f32, tag="sym_t")
                nc.vector.memset(sym_t[:, :], 0.0)
                nc.sync.dma_start(out=sym_t[0:127, :], in_=symdn[1:128, :])

                l0_t = sc_pool.tile([L, CHUNK], f32, tag="l0")
                l2_t = sc_pool.tile([L, CHUNK], f32, tag="l2")
                nc.vector.tensor_tensor(
                    out=l0_t[:, :], in0=symdn[:, :], in1=m0_t[:, sl], op=ALU.add
                )
                nc.vector.tensor_tensor(
                    out=l2_t[:, :], in0=sym_t[:, :], in1=m2_t[:, sl], op=ALU.add
                )
                e0_t = sc_pool.tile([L, CHUNK], f32, tag="e0")
                e1_t = sc_pool.tile([L, CHUNK], f32, tag="e1")
                e2_t = sc_pool.tile([L, CHUNK], f32, tag="e2")
                nc.scalar.activation(e0_t[:, :], l0_t[:, :], AF.Exp)
                nc.scalar.activation(e1_t[:, :], a_t[:, :], AF.Exp)
                nc.scalar.activation(e2_t[:, :], l2_t[:, :], AF.Exp)
                den_t = sc_pool.tile([L, CHUNK], f32, tag="den")
                nc.vector.tensor_tensor(
                    out=den_t[:, :], in0=e0_t[:, :], in1=e1_t[:, :], op=ALU.add
                )
                nc.vector.tensor_tensor(
                    out=den_t[:, :], in0=den_t[:, :], in1=e2_t[:, :], op=ALU.add
                )
                r_t = sc_pool.tile([L, CHUNK], f32, tag="r")
                nc.vector.reciprocal(r_t[:, :], den_t[:, :])
                w1c = sc_pool.tile([L, CHUNK], f32, tag="w1c")
                w0c = sc_pool.tile([L, CHUNK], f32, tag="w0c")
                w2c = sc_pool.tile([L, CHUNK], f32, tag="w2c")
                nc.vector.tensor_tensor(
                    out=w1c[:, :], in0=e1_t[:, :], in1=r_t[:, :], op=ALU.mult
                )
                nc.vector.tensor_tensor(
                    out=w0c[:, :], in0=e0_t[:, :], in1=r_t[:, :], op=ALU.mult
                )
                nc.vector.tensor_tensor(
                    out=w2c[:, :], in0=e2_t[:, :], in1=r_t[:, :], op=ALU.mult
                )
                w0up = sc_pool.tile([L, CHUNK], f32, tag="w0up")
                w2dn = sc_pool.tile([L, CHUNK], f32, tag="w2dn")
                nc.vector.memset(w0up[:, :], 0.0)
                nc.vector.memset(w2dn[:, :], 0.0)
                nc.sync.dma_start(out=w0up[0:127, :], in_=w0c[1:128, :])
                nc.sync.dma_start(out=w2dn[1:128, :], in_=w2c[0:127, :])

                # ---- deferred A^T + combine of previous chunk ----
                if prev is not None:
                    emit_combine(prev)
                prev = (b0, s_t, w1c, w0up, w2dn, c)

            emit_combine(prev)

    nc.compile()
    return nc


_NC_CACHE = {}


def _get_nc():
    if "nc" not in _NC_CACHE:
        _NC_CACHE["nc"] = _build_nc()
    return _NC_CACHE["nc"]


def _host_inputs(sentence, size, W):
    sentence = np.ascontiguousarray(np.asarray(sentence, dtype=np.float32))
    size = np.asarray(size).astype(np.int64)
    W = np.asarray(W, dtype=np.float32)

    wsym = (0.5 * 16.0) * (W + W.T)   # x16 for fp8 range; 1/16 folded into dots
    w_dr = np.zeros((128, 2, DP), dtype=ml_dtypes.float8_e4m3)
    w_dr[:, 0, 0:D] = wsym[0:128, :].astype(ml_dtypes.float8_e4m3)
    w_dr[:, 1, 0:D] = wsym[128:256, :].astype(ml_dtypes.float8_e4m3)
    w_tl = np.zeros((44, DP), dtype=ml_dtypes.float8_e4m3)
    w_tl[:, 0:D] = wsym[256:300, :].astype(ml_dtypes.float8_e4m3)

    ident = np.eye(128, dtype=ml_dtypes.bfloat16)
    I0 = np.eye(128, dtype=np.float32)
    Iup = np.zeros((128, 128), np.float32)
    Iup[np.arange(127), np.arange(1, 128)] = 1.0
    Idn = np.zeros((128, 128), np.float32)
    Idn[np.arange(1, 128), np.arange(127)] = 1.0
    imask = np.ascontiguousarray(
        np.concatenate([I0, Iup, Idn], axis=1).astype(np.uint8)
    )

    pos = np.arange(L, dtype=np.int64)[:, None]
    m0 = np.where(pos < size[None, :], 0.0, NEG).astype(np.float32)
    m0[0, :] = NEG
    m2 = np.where(pos < np.clip(size - 1, 0, None)[None, :], 0.0, NEG).astype(
        np.float32
    )
    m2[L - 1, :] = NEG

    in_maps = []
    for c in range(NCORES):
        bsl = slice(c * BC, (c + 1) * BC)
        in_maps.append(
            {
                "s": np.ascontiguousarray(sentence[:, bsl, :]),
                "m0": np.ascontiguousarray(m0[:, bsl]),
                "m2": np.ascontiguousarray(m2[:, bsl]),
                "wdr": w_dr,
                "wtl": w_tl,
                "ident": ident,
                "imask": imask,
            }
        )
    return in_maps


def kernel(sentence, size, W):
    nc = _get_nc()
    in_maps = _host_inputs(sentence, size, W)
    res = run_bass_kernel_spmd(nc, in_maps, core_ids=list(range(NCORES)))
    out = np.concatenate([res.results[c]["o"] for c in range(NCORES)], axis=1)
    return out.astype(np.float32)


def _install_ntff_hook():
    """Register the axon NTFF profiling hook that this container's boot
    skipped (antenv.axon_hooks module absent). Mirrors
    trn_agent_boot/trn_boot.py:_ntff_profile_via_ctypes."""
    try:
        from antenv.axon_hooks import get_axon_ntff_profile_hook  # noqa: F401

        return
    except ImportError:
        pass
    import contextlib
    import ctypes
    import types

    so_path = "/opt/axon/libaxon_pjrt.so"
    lib = ctypes.CDLL(so_path)
    if not hasattr(lib, "axon_start_nrt_profile"):
        return
    lib.axon_start_nrt_profile.argtypes = [
        ctypes.POINTER(ctypes.c_int64),
        ctypes.c_size_t,
    ]
    lib.axon_start_nrt_profile.restype = ctypes.c_int64
    lib.axon_stop_nrt_profile.argtypes = [ctypes.c_char_p]
    lib.axon_stop_nrt_profile.restype = ctypes.c_int64

    @contextlib.contextmanager
    def _hook(output_dir, device_ids):
        import jax

        jax.devices()
        if device_ids:
            ids = (ctypes.c_int64 * len(device_ids))(*device_ids)
            rc = lib.axon_start_nrt_profile(ids, len(device_ids))
        else:
            rc = lib.axon_start_nrt_profile(None, 0)
        if rc != 0:
            raise RuntimeError(f"axon_start_nrt_profile rc={rc}")
        try:
            yield
        finally:
            n = lib.axon_stop_nrt_profile(str(output_dir).encode())
            print(f"ntff capture: {n} file(s) -> {output_dir}")

    mod = types.ModuleType("antenv.axon_hooks")
    mod.get_axon_ntff_profile_hook = lambda: _hook
    mod.set_axon_ntff_profile_hook = lambda h: None
    import antenv

    sys.modules["antenv.axon_hooks"] = mod
    antenv.axon_hooks = mod


def run_traced(sentence, size, W):
    """Like kernel(), but also returns (exec_time_ns, profile_json path)."""
    _install_ntff_hook()
    nc = _get_nc()
    in_maps = _host_inputs(sentence, size, W)
    res = run_bass_kernel_spmd(
        nc, in_maps, core_ids=list(range(NCORES)), trace=True, trace_cores=[0]
    )
    out = np.concatenate([res.results[c]["o"] for c in range(NCORES)], axis=1)
    return out.astype(np.float32), res.exec_time_ns, res.profile_json


if __name__ == "__main__":
    rng = np.random.default_rng(0)
    s = rng.standard_normal((L, B, D)).astype(np.float32)
    sz = rng.integers(0, L, size=(B,)).astype(np.int32)
    W = (rng.standard_normal((D, D)) / np.sqrt(D)).astype(np.float32)
    out = kernel(s, sz, W)
    print("out", out.shape, out.dtype, np.abs(out).max())
